# revision 1
# baseline (speedup 1.0000x reference)
"""DenseCapsule routing kernel for Trainium2 (Bass/Tile), 8-core data-parallel.

Problem: x [64, 8192, 8], W [8, 160], bias [160] ->
  x_hat = (x @ W + bias).reshape(64, 8192, 10, 16)
  3 dynamic-routing iterations (softmax over out_num=10, weighted sum over
  in_num=8192, squash over the 10-axis, agreement update), return
  ||outputs||_2 over out_dim -> [64, 10].

Key algebra (x_hat is never materialized):
  s[b,j,:]  = y[b,j,:] @ Wr[:,j,:]          with y = c^T @ x_aug   ([10,9] tiny)
  b_logits  = x_aug @ vhat_acc^T            vhat accumulates over iterations
  softmax:  c = exp(b)/Z; the 1/Z[i] is folded into x_aug (x' = x_aug/Z)
  exp without max-subtraction is safe: |b| <= ~45 << 88 (fp32 exp overflow).

Sharding: batch 64 -> 8 cores x 8 batches. Within a core, the 8 batches are
stacked on the free/partition dims ((b,d)=72 rows, (b,j)=80 rows) so every
engine op covers all 8 batches at once. All per-iteration data movement is
engine work (PE/DVE/ACT); there are no per-iteration DMAs.
"""

from contextlib import ExitStack

import numpy as np

import concourse.bacc as bacc
import concourse.bass as bass
import concourse.mybir as mybir
import concourse.tile as tile
import concourse.bass_utils as bass_utils

f32 = mybir.dt.float32

P = 128          # SBUF partitions
NH = 64          # i-chunks per batch (8192 / 128)
NB = 8           # batches per core
D = 8            # input capsule dim
DA = 9           # augmented (+ ones column)
J = 10           # out_num
KD = 16          # out_dim
KT = NB * DA     # 72 rows: 0..63 = (b,d) as b*8+d, 64..71 = ones-row per b
BJ = NB * J      # 80 rows (b, j)
IN = 8192
N_CORES = 8
EPS = 1e-8

# b-update wave geometry: 8 chunks per wave, 4 chunks per PSUM bank.
WAVE_CHUNKS = 8
CHUNKS_PER_BANK = 4
BANKS_PER_WAVE = 2
N_WAVES = NH // WAVE_CHUNKS  # 8


def _build_nc():
    nc = bacc.Bacc(
        "TRN2", target_bir_lowering=False, debug=False, num_devices=N_CORES
    )

    x_d = nc.dram_tensor("x", [NB, IN, D], f32, kind="ExternalInput").ap()
    w_d = nc.dram_tensor("W", [D, J * KD], f32, kind="ExternalInput").ap()
    bias_d = nc.dram_tensor("bias", [J * KD], f32, kind="ExternalInput").ap()
    out_d = nc.dram_tensor("out", [BJ, 1], f32, kind="ExternalOutput").ap()

    # ---- structural constants embedded in the NEFF ----
    ident_np = np.eye(P, dtype=np.float32)
    blkdup_np = np.zeros((BJ, BJ), dtype=np.float32)
    for b in range(NB):
        blkdup_np[b * J:(b + 1) * J, b * J:(b + 1) * J] = 1.0
    # blkones[(b,d) row, (b,j) col] = 1 iff same b; rows are b*9+d, d<=8
    blkones_np = np.zeros((KT, BJ), dtype=np.float32)
    for b in range(NB):
        blkones_np[b * DA:(b + 1) * DA, b * J:(b + 1) * J] = 1.0
    # cBLKY[(b,j), (b',d)] = 1 iff b' == b  (mask for the s computation)
    blky_np = np.zeros((BJ, KT), dtype=np.float32)
    for b in range(NB):
        blky_np[b * J:(b + 1) * J, b * DA:(b + 1) * DA] = 1.0
    # cJ10[j', (b,j)] = 1 iff j' == j  (selection for the Wr build matmul)
    cj10_np = np.zeros((J, BJ), dtype=np.float32)
    for b in range(NB):
        for j in range(J):
            cj10_np[j, b * J + j] = 1.0
    # REP[d, r] replicates vT rows into the (b,d)-row space:
    #   r = b*8+d' -> row d' (d' < 8);  r = 64+b -> row 8 (the bias/t row)
    rep_np = np.zeros((DA, KT), dtype=np.float32)
    for b in range(NB):
        for d in range(DA):
            rep_np[d, b * DA + d] = 1.0
    ident_d = nc.inline_tensor(ident_np, "ident128").ap()
    import ml_dtypes
    identbf_d = nc.inline_tensor(
        ident_np.astype(ml_dtypes.bfloat16), "ident128bf"
    ).ap()
    blkdup_d = nc.inline_tensor(blkdup_np, "blkdup80").ap()
    blkones_d = nc.inline_tensor(blkones_np, "blkones72").ap()
    rep_d = nc.inline_tensor(rep_np, "rep9x72").ap()
    blky_d = nc.inline_tensor(blky_np, "blky80").ap()
    cj10_d = nc.inline_tensor(cj10_np, "cj10").ap()

    with tile.TileContext(nc) as tc, ExitStack() as ctx:
        sbp = ctx.enter_context(tc.tile_pool(name="sbp", bufs=1))
        nti = [0]

        def T(shape, name=None):
            if name is None:
                nti[0] += 1
                name = f"t{nti[0]}"
            return sbp.tile(shape, f32, name=name, tag=name)

        # ----- persistent SBUF tensors -----
        def Tbf(shape, name=None):
            if name is None:
                nti[0] += 1
                name = f"t{nti[0]}"
            return sbp.tile(shape, mybir.dt.bfloat16, name=name, tag=name)

        x_main = T([P, NB, NH, D])     # raw x, contiguous per batch
        x_split = Tbf([P, NH, 2, NB, DA])  # [hi | lo] bf16 split of x_aug
        c_stack = Tbf([P, NH, NB, J])  # softmax weights c = e * (1/Z)
        e_stack = Tbf([P, NH, NB, J])  # exp(b) = exp(b_hi) * exp(b_lo)
        e2 = Tbf([P, NH, 2, NB, J])    # exp(b_hi), exp(b_lo) halves
        Zr_bf = Tbf([P, NH, NB])       # bf16 copy of 1/Z
        xT = Tbf([KT, NH, P])          # bf16 x_aug^T; rows b*8+d, ones at 64+b
        Zs = T([P, NH, NB])            # row sums of e
        Zr = T([P, NH, NB])            # 1/Z
        Zscr = T([P, NH, NB])          # recip scratch
        Wr = T([BJ, DA, KD])           # Wr[(b,j), d, k] = W_aug[d, j*16+k]
        WrBIG = T([BJ, NB, DA, KD])    # cBLKY-masked, b'-expanded Wr
        W10 = T([J, DA, KD])           # W10[j, d, k] = W_aug[d, j*16+k]
        cBLKY = T([BJ, KT])            # same-batch mask over y columns
        cJ10 = T([J, BJ])              # j-selection matrix
        blkv = Tbf([KT, 2, BJ])        # block-diag vhat_acc: [hi | lo] bf16
        blkM = T([KT, BJ])             # f32 masked vdup
        vacc = T([BJ, DA])
        cI = T([P, P])                 # identity for PE transpose (f32)
        cIbf = Tbf([P, P])             # identity for bf16 transposes
        tenth80 = T([P, BJ])           # 1/J constant block for iter-0 y
        cB80 = T([BJ, BJ])             # block-dup matrix (squash)
        cBLK = T([KT, BJ])             # blkones mask
        cREP = T([DA, KT])             # vT row-replication matrix

        # ----- input x: 8 fully-contiguous DMAs across both HW queues -----
        for b in range(NB):
            eng = nc.sync if b % 2 == 0 else nc.scalar
            eng.dma_start(
                x_main[:, b, :, :],
                x_d[b].rearrange("(p h) d -> p h d", p=P),
            )

        # ----- constants (scalar HW queue; x owns the sync queue) -----
        nc.scalar.dma_start(cI[:, :], ident_d[:, :])
        nc.scalar.dma_start(cIbf[:, :], identbf_d[:, :])
        nc.vector.memset(tenth80[:, :], 1.0 / J)
        nc.scalar.dma_start(cB80[:, :], blkdup_d[:, :])
        nc.scalar.dma_start(cBLK[:, :], blkones_d[:, :])
        nc.scalar.dma_start(cREP[:, :], rep_d[:, :])
        nc.scalar.dma_start(cBLKY[:, :], blky_d[:, :])
        nc.scalar.dma_start(cJ10[:, :], cj10_d[:, :])

        # W10[j, d, k] = W_aug[d, j*16+k]: 2 DMAs; Wr comes from a matmul
        nc.scalar.dma_start(
            W10[:, 0:D, :],
            bass.AP(tensor=w_d.tensor, offset=0,
                    ap=[[KD, J], [J * KD, D], [1, KD]]),
        )
        nc.scalar.dma_start(
            W10[:, D, :],
            bass.AP(tensor=bias_d.tensor, offset=0, ap=[[KD, J], [1, KD]]),
        )

        # ----- bf16 split of x_aug (h-outer, hi/lo interleaved) -----
        QH4 = NH // 4
        for q4 in range(4):
            hq = q4 * QH4
            nc.vector.tensor_copy(
                x_split[:, hq:hq + QH4, 0, :, 0:D].transpose([0, 2, 1, 3]),
                x_main[:, :, hq:hq + QH4, :],
            )
            nc.gpsimd.memset(x_split[:, hq:hq + QH4, 0, :, D], 1.0)
        nc.gpsimd.tensor_sub(
            x_split[:, :, 1, :, 0:D].transpose([0, 2, 1, 3]),
            x_main[:, :, :, :],
            x_split[:, :, 0, :, 0:D].transpose([0, 2, 1, 3]),
        )
        nc.gpsimd.memset(x_split[:, :, 1, :, D], 0.0)

        # ----- xT build: 64 PE transposes [128, 72] -> [72, 128] -----
        # rows are b*9+d with d<=8 (the d=8 ones column included)
        with tc.tile_pool(name="tpp", bufs=2, space="PSUM") as tpp:
            for w in range(0, NH, 4):
                tp = tpp.tile(
                    [KT, 4, P], mybir.dt.bfloat16, tag="tp", name=f"tp_{w}"
                )
                for q in range(4):
                    h = w + q
                    nc.tensor.transpose(
                        tp[:, q, :],
                        x_split[:, h, 0, :, :],
                        cIbf[:, :],
                    )
                if (w // 4) % 2 == 0:
                    nc.vector.tensor_copy(xT[:, w:w + 4, :], tp[:, :, :])
                else:
                    nc.scalar.copy(xT[:, w:w + 4, :], tp[:, :, :])

        # ----- routing iterations -----
        with (
            tc.tile_pool(name="bwp", bufs=2, space="PSUM") as bwp,
            tc.tile_pool(name="yp", bufs=2, space="PSUM") as yp,
        ):
            wr_ps = yp.tile([BJ, DA, KD], f32, tag="ypsum", name="wr_ps")
            nc.tensor.matmul(
                wr_ps[:, :, :], cJ10[:, :],
                W10[:, :, :], start=True, stop=True,
            )
            nc.vector.tensor_copy(Wr[:, :, :], wr_ps[:, :, :])
            nc.vector.tensor_mul(
                WrBIG[:, :, :, :],
                cBLKY[:, :].rearrange("p (b d) -> p b d", d=DA)
                .unsqueeze(3).broadcast_to((BJ, NB, DA, KD)),
                Wr[:, :, :].unsqueeze(1).broadcast_to((BJ, NB, DA, KD)),
            )

            for m in range(3):
                if m == 0:
                    pass  # uniform c handled by the colsum shortcut below
                else:
                    # ---- b-update: one N=160 matmul per chunk ([hi|lo]),
                    # exp of both halves, product, Z — wave-pipelined ----
                    for w0 in range(0, NH, 6):
                        cnt = min(6, NH - w0)
                        nbank = (cnt + 2) // 3
                        bw = bwp.tile(
                            [P, 2, 512], f32, tag="bw", name=f"bw_{m}_{w0}",
                        )
                        per_bank = cnt // nbank
                        for c in range(cnt):
                            h = w0 + c
                            off = (c % per_bank) * 2 * BJ
                            nc.tensor.matmul(
                                bw[:, c // per_bank, off:off + 2 * BJ],
                                xT[:, h, :],
                                blkv[:, :, :],
                                start=True, stop=True,
                            )
                        nc.scalar.activation(
                            e2[:, w0:w0 + cnt, :, :, :].rearrange(
                                "p (a c) s b j -> p a c (s b j)", a=nbank
                            ),
                            bw[:, :, 0:per_bank * 2 * BJ].rearrange(
                                "p a (c e) -> p a c e", e=2 * BJ
                            ),
                            mybir.ActivationFunctionType.Exp,
                        )
                        nc.vector.tensor_mul(
                            e_stack[:, w0:w0 + cnt, :, :],
                            e2[:, w0:w0 + cnt, 0, :, :],
                            e2[:, w0:w0 + cnt, 1, :, :],
                        )
                        nc.vector.reduce_sum(
                            Zs[:, w0:w0 + cnt, :],
                            e_stack[:, w0:w0 + cnt, :, :],
                            axis=mybir.AxisListType.X,
                        )


                y_full = T([BJ, KT], name=f"y_full_{m}")
                if m == 0:
                    # y1 = (1/J) * colsum(x_aug), identical for every j
                    cs1 = T([P, NB, D], name="cs1")
                    nc.vector.reduce_sum(
                        cs1[:, :, :],
                        x_main[:, :, :, :].transpose([0, 1, 3, 2]),
                        axis=mybir.AxisListType.X,
                    )
                    y0_ps = yp.tile([BJ, NB * D], f32, tag="ypsum", name="y0")
                    nc.tensor.matmul(
                        y0_ps[:, :], tenth80[:, :],
                        cs1[:, :, :], start=True, stop=True,
                    )
                    nc.vector.tensor_copy(
                        y_full[:, :].rearrange(
                            "p (b d) -> p b d", d=DA
                        )[:, :, 0:D],
                        y0_ps[:, :],
                    )
                    nc.vector.memset(
                        y_full[:, :].rearrange(
                            "p (b d) -> p b d", d=DA
                        )[:, :, D],
                        IN / J,
                    )
                else:
                    # recip/cast/c in 4 h-parts, pipelined with the y matmuls
                    y_ps = yp.tile([BJ, 2, KT], f32, tag="ypsum", name=f"y_{m}")
                    QH = NH // 4
                    for q in range(4):
                        h0 = q * QH
                        nc.vector.reciprocal_approx_accurate(
                            Zr[:, h0:h0 + QH, :].rearrange("p h b -> p (h b)"),
                            Zs[:, h0:h0 + QH, :].rearrange("p h b -> p (h b)"),
                            Zscr[:, h0:h0 + QH, :].rearrange("p h b -> p (h b)"),
                        )
                        nc.scalar.copy(
                            Zr_bf[:, h0:h0 + QH, :], Zr[:, h0:h0 + QH, :]
                        )
                        nc.vector.tensor_mul(
                            c_stack[:, h0:h0 + QH, :, :],
                            e_stack[:, h0:h0 + QH, :, :],
                            Zr_bf[:, h0:h0 + QH, :].unsqueeze(3)
                            .broadcast_to((P, QH, NB, J)),
                        )
                        for h in range(h0, h0 + QH):
                            nc.tensor.matmul(
                                y_ps[:, :, :],
                                c_stack[:, h, :, :],
                                x_split[:, h, :, :, :],
                                start=(h == 0), stop=(h == NH - 1),
                            )
                    nc.scalar.copy(y_full[:, :], y_ps[:, 0, :])
                    nc.vector.tensor_add(
                        y_full[:, :], y_full[:, :], y_ps[:, 1, :]
                    )

                # ---- s[(b,j), k] = sum_{b',d} y[(b,j), (b',d)] WrBIG[...]
                sBIG = T([BJ, NB, DA, KD], name=f"sBIG_{m}")
                s_sb = T([BJ, KD], name=f"s_sb_{m}")
                nc.vector.tensor_mul(
                    sBIG[:, :, :, :],
                    y_full[:, :].rearrange("p (b d) -> p b d", d=DA)
                    .unsqueeze(3).broadcast_to((BJ, NB, DA, KD)),
                    WrBIG[:, :, :, :],
                )
                nc.vector.reduce_sum(
                    s_sb[:, :],
                    sBIG[:, :, :, :].transpose([0, 3, 1, 2]),
                    axis=mybir.AxisListType.XY,
                )

                # ---- squash over j: nsq[(b,j), k] = sum_j' s[(b,j'), k]^2 ----
                s2 = T([BJ, KD], name=f"s2_{m}")
                nc.scalar.square(s2[:, :], s_sb[:, :])
                nsq_ps = yp.tile([BJ, KD], f32, tag="ypsum", name=f"nsq_{m}")
                nc.tensor.matmul(
                    nsq_ps[:, :], cB80[:, :], s2[:, :], start=True, stop=True
                )
                nrm = T([BJ, KD], name=f"nrm_{m}")
                nc.scalar.sqrt(nrm[:, :], nsq_ps[:, :])
                a1 = T([BJ, KD], name=f"a1_{m}")
                nc.vector.tensor_scalar_add(a1[:, :], nsq_ps[:, :], 1.0)
                a2 = T([BJ, KD], name=f"a2_{m}")
                nc.vector.tensor_scalar_add(a2[:, :], nrm[:, :], EPS)
                a3 = T([BJ, KD], name=f"a3_{m}")
                nc.vector.tensor_mul(a3[:, :], a1[:, :], a2[:, :])
                rr = T([BJ, KD], name=f"rr_{m}")
                rscr = T([BJ, KD], name=f"rscr_{m}")
                nc.vector.reciprocal_approx_accurate(rr[:, :], a3[:, :], rscr[:, :])
                scale = T([BJ, KD], name=f"scale_{m}")
                nc.vector.tensor_mul(scale[:, :], nsq_ps[:, :], rr[:, :])
                o_sb = T([BJ, KD], name=f"o_{m}")
                nc.vector.tensor_mul(o_sb[:, :], s_sb[:, :], scale[:, :])

                if m < 2:
                    # ---- vhat[(b,j), d] = sum_k Wr[(b,j), d, k] * o[(b,j), k]
                    v_tmp = T([BJ, DA, KD], name=f"v_tmp_{m}")
                    nc.vector.tensor_mul(
                        v_tmp[:, :, :],
                        o_sb[:, :].unsqueeze(1).broadcast_to((BJ, DA, KD)),
                        Wr[:, :, :],
                    )
                    if m == 0:
                        nc.vector.reduce_sum(
                            vacc[:, :], v_tmp[:, :, :], axis=mybir.AxisListType.X
                        )
                    else:
                        v_cur = T([BJ, DA], name=f"v_cur_{m}")
                        nc.vector.reduce_sum(
                            v_cur[:, :], v_tmp[:, :, :], axis=mybir.AxisListType.X
                        )
                        nc.vector.tensor_add(vacc[:, :], vacc[:, :], v_cur[:, :])
                    # blkv = blkones (*) REP-replicated vacc^T   (no DMAs)
                    vT_ps = yp.tile([DA, BJ], f32, tag="ypsum", name=f"vT_{m}")
                    nc.tensor.transpose(
                        vT_ps[:, :], vacc[:, :], cI[0:BJ, 0:BJ]
                    )
                    vT_sb = T([DA, BJ], name=f"vT_sb_{m}")
                    nc.vector.tensor_copy(vT_sb[:, :], vT_ps[:, :])
                    vdup_ps = yp.tile([KT, BJ], f32, tag="ypsum", name=f"vdup_{m}")
                    nc.tensor.matmul(
                        vdup_ps[:, :], cREP[:, :], vT_sb[:, :],
                        start=True, stop=True,
                    )
                    nc.vector.tensor_mul(blkM[:, :], cBLK[:, :], vdup_ps[:, :])
                    nc.gpsimd.tensor_copy(blkv[:, 0, :], blkM[:, :])
                    nc.gpsimd.tensor_sub(
                        blkv[:, 1, :], blkM[:, :], blkv[:, 0, :]
                    )
                else:
                    # ---- final lengths ||o_b[j, :]|| ----
                    osq = T([BJ, KD], name="osq")
                    nc.scalar.square(osq[:, :], o_sb[:, :])
                    lsum = T([BJ, 1], name="lsum")
                    nc.vector.reduce_sum(
                        lsum[:, :], osq[:, :], axis=mybir.AxisListType.X
                    )
                    lnorm = T([BJ, 1], name="lnorm")
                    nc.scalar.sqrt(lnorm[:, :], lsum[:, :])
                    nc.sync.dma_start(out_d[:, :], lnorm[:, :])

    nc.compile()
    return nc


_NC_CACHE = None


def _get_nc():
    global _NC_CACHE
    if _NC_CACHE is None:
        _NC_CACHE = _build_nc()
    return _NC_CACHE


def kernel(x, W, bias):
    x = np.ascontiguousarray(np.asarray(x, dtype=np.float32))
    W = np.ascontiguousarray(np.asarray(W, dtype=np.float32))
    bias = np.ascontiguousarray(np.asarray(bias, dtype=np.float32))
    B = x.shape[0]
    per = B // N_CORES

    nc = _get_nc()
    in_maps = [
        {"x": x[i * per:(i + 1) * per], "W": W, "bias": bias}
        for i in range(N_CORES)
    ]
    res = bass_utils.run_bass_kernel_spmd(
        nc, in_maps, core_ids=list(range(N_CORES))
    )
    outs = [r["out"].reshape(NB, J) for r in res.results]
    return np.concatenate(outs, axis=0)


if __name__ == "__main__":
    rng = np.random.default_rng(0)
    x = rng.standard_normal((64, IN, D), dtype=np.float32)
    W = (rng.standard_normal((D, J * KD)) / np.sqrt(D)).astype(np.float32)
    bias = (rng.standard_normal(J * KD) * 0.01).astype(np.float32)
    out = kernel(x=x, W=W, bias=bias)
    print(out.shape, out[0])



# revision 70
# speedup vs baseline: 1.5357x; 1.5357x over previous
"""DenseCapsule routing kernel for Trainium2 (Bass/Tile), 8-core data-parallel.

Problem: x [64, 8192, 8], W [8, 160], bias [160] ->
  x_hat = (x @ W + bias).reshape(64, 8192, 10, 16)
  3 dynamic-routing iterations (softmax over out_num=10, weighted sum over
  in_num=8192, squash over the 10-axis, agreement update), return
  ||outputs||_2 over out_dim -> [64, 10].

Key algebra (x_hat is never materialized):
  s[b,j,:]  = y[b,j,:] @ Wr[:,j,:]      with y = c^T @ x_aug  (tiny [10,9])
  b_logits  = x_aug @ vhat_acc^T        vhat accumulates over iterations
  softmax over j=10: c = e * (1/Z), Z via a bf16 pair-tree
  sqrt via gpsimd pow so ACT stays on ONE function table (exp only).

Precision: x_aug, c in single bf16; vhat_acc split hi/lo bf16 and the two
halves accumulated in PSUM by two back-to-back matmuls (f32 accumulate);
m0 colsum and the s/squash/vhat path in f32. End-to-end ~7e-4 rel err.

Sharding: batch 64 -> 8 cores x 8 batches. Row space for [80, *] tensors is
(j, b) = j*8+b; xT rows are (b, d) = b*9+d.
"""

from contextlib import ExitStack

import numpy as np

import concourse.bacc as bacc
import concourse.bass as bass
import concourse.mybir as mybir
import concourse.tile as tile
import concourse.bass_utils as bass_utils

f32 = mybir.dt.float32
bf16 = mybir.dt.bfloat16
AF = mybir.ActivationFunctionType
ALU = mybir.AluOpType

_DEBUG_TAPS = False

P = 128          # SBUF partitions
NH = 64          # i-chunks per batch (8192 / 128)
NB = 8           # batches per core
D = 8            # input capsule dim
DA = 9           # augmented (+ ones column)
J = 10           # out_num
KD = 16          # out_dim
KT = NB * DA     # 72 rows (b, d)
BJ = NB * J      # 80 rows (j, b) = j*8+b
IN = 8192
N_CORES = 8


def _build_nc():
    nc = bacc.Bacc(
        "TRN2", target_bir_lowering=False, debug=False, num_devices=N_CORES
    )

    x_d = nc.dram_tensor("x", [NB, IN, D], f32, kind="ExternalInput").ap()
    w_d = nc.dram_tensor("W", [D, J * KD], f32, kind="ExternalInput").ap()
    bias_d = nc.dram_tensor("bias", [J * KD], f32, kind="ExternalInput").ap()
    out_d = nc.dram_tensor("out", [BJ, 1], f32, kind="ExternalOutput").ap()
    dbg_d = None
    dbg2_d = None
    if _DEBUG_TAPS:
        dbg_d = nc.dram_tensor(
            "dbg", [2, BJ, DA], f32, kind="ExternalOutput"
        ).ap()
        dbg2_d = nc.dram_tensor(
            "dbg2", [4, BJ, KD * KT], f32, kind="ExternalOutput"
        ).ap()

    # ---- structural constants, packed into one f32 block + bf16 identity ----
    import ml_dtypes
    identbf_np = np.eye(P, dtype=np.float32).astype(ml_dtypes.bfloat16)

    # column offsets in the packed f32 const block
    C_I80, C_B80, C_BLK, C_BLKY, C_J10, C_REP = 0, 80, 160, 240, 312, 392
    CPACK_W = C_REP + KT
    cpack_np = np.zeros((P, CPACK_W), dtype=np.float32)
    cpack_np[0:BJ, C_I80:C_I80 + BJ] = np.eye(BJ, dtype=np.float32)
    for j in range(J):
        for b in range(NB):
            for j2 in range(J):
                # cB80[(j,b), (j',b')] = 1 iff b == b'
                cpack_np[j * NB + b, C_B80 + j2 * NB + b] = 1.0
    for b in range(NB):
        for d in range(DA):
            for j in range(J):
                # cBLK[(b,d), (j,b')] = 1 iff b == b'
                cpack_np[b * DA + d, C_BLK + j * NB + b] = 1.0
    for j in range(J):
        for b in range(NB):
            # cBLKY[(j,b), (b',d)] = 1 iff b' == b
            cpack_np[j * NB + b, C_BLKY + b * DA:C_BLKY + (b + 1) * DA] = 1.0
    for j in range(J):
        for b in range(NB):
            # cJ10[j', (j,b)] = 1 iff j' == j
            cpack_np[j, C_J10 + j * NB + b] = 1.0
    for b in range(NB):
        for d in range(DA):
            # cREP[d, (b,d')] = 1 iff d' == d
            cpack_np[d, C_REP + b * DA + d] = 1.0

    identbf_d = nc.inline_tensor(identbf_np, "identbf").ap()
    cpack_d = nc.inline_tensor(cpack_np, "cpack").ap()

    with tile.TileContext(nc) as tc, ExitStack() as ctx:
        sbp = ctx.enter_context(tc.tile_pool(name="sbp", bufs=1))

        def T(shape, name, dt=f32):
            return sbp.tile(shape, dt, name=name, tag=name)

        # ----- persistent SBUF tensors -----
        x_main = T([P, NB, NH, D], "x_main")          # raw DMA staging
        x_bf = T([P, NH, NB, DA], "x_bf", bf16)       # bf16 x_aug (ones col)
        xT = T([KT, NH, P], "xT", bf16)               # x_aug^T per chunk
        cIbf = T([P, P], "cIbf", bf16)                # identity (transposes)
        cpack = T([P, CPACK_W], "cpack")              # packed f32 constants
        cI80 = cpack[0:BJ, C_I80:C_I80 + BJ]
        cB80 = cpack[0:BJ, C_B80:C_B80 + BJ]
        cBLK = cpack[0:KT, C_BLK:C_BLK + BJ]
        cBLKY = cpack[0:BJ, C_BLKY:C_BLKY + KT]
        cJ10 = cpack[0:J, C_J10:C_J10 + BJ]
        cREP = cpack[0:DA, C_REP:C_REP + KT]
        W10 = T([J, DA, KD], "W10")                   # W_aug per j
        Wr = T([BJ, DA, KD], "Wr")                    # Wr[(j,b), d, k]
        WrBIGt = T([BJ, KD, NB, DA], "WrBIGt")        # masked, k-outer (f32)
        vacc = T([BJ, DA], "vacc")
        halfs = T([BJ, KD], "halfs")                  # 0.5 (gpsimd pow -> sqrt)
        part0 = T([P, NB, D], "part0")                # m0 f32 colsum partials
        onesF = T([P, 1], "onesF")                    # f32 ones column
        tenth80 = T([1, BJ], "tenth80")               # 0.1 expander row
        y0row = T([1, KT], "y0row")                   # m0 colsum row

        e_st = [None, T([P, NH, J, NB], "e1", bf16), T([P, NH, J, NB], "e2", bf16)]
        c_st = [None, T([P, NH, J, NB], "c1", bf16), T([P, NH, J, NB], "c2", bf16)]
        u5_t = [None, T([P, NH, 5, NB], "u5_1", bf16), T([P, NH, 5, NB], "u5_2", bf16)]
        v2_t = [None, T([P, NH, 2, NB], "v2_1", bf16), T([P, NH, 2, NB], "v2_2", bf16)]
        w1_t = [None, T([P, NH, NB], "w1_1", bf16), T([P, NH, NB], "w1_2", bf16)]
        Z_t = [None, T([P, NH, NB], "Z_1"), T([P, NH, NB], "Z_2")]
        Zr_t = [None, T([P, NH, NB], "Zr_1"), T([P, NH, NB], "Zr_2")]
        Zrb_t = [None, T([P, NH, NB], "Zrb_1", bf16), T([P, NH, NB], "Zrb_2", bf16)]
        blkv_t = [T([KT, 2, BJ], "blkv0", bf16), T([KT, 2, BJ], "blkv1", bf16)]

        # ----- input x: 8 contiguous per-batch DMAs over both HW queues -----
        for b in range(NB):
            eng = nc.sync if b % 2 == 0 else nc.scalar
            eng.dma_start(
                x_main[:, b, :, :],
                x_d[b].rearrange("(p h) d -> p h d", p=P),
            )

        # ----- constants -----
        nc.sync.dma_start(cpack[:, :], cpack_d[:, :])
        nc.scalar.dma_start(cIbf[:, :], identbf_d[:, :])
        # W10[j, d, k] = W_aug[d, j*16+k]
        nc.scalar.dma_start(
            W10[:, 0:D, :],
            bass.AP(tensor=w_d.tensor, offset=0,
                    ap=[[KD, J], [J * KD, D], [1, KD]]),
        )
        nc.scalar.dma_start(
            W10[:, D, :],
            bass.AP(tensor=bias_d.tensor, offset=0, ap=[[KD, J], [1, KD]]),
        )

        nc.gpsimd.memset(halfs[:, :], 0.5)
        nc.gpsimd.memset(onesF[:, :], 1.0)
        nc.gpsimd.memset(tenth80[:, :], 1.0 / J)
        nc.gpsimd.memset(x_bf[:, :, :, D], 1.0)
        # warm the gpsimd pow library during the DMA wait (the ext-isa
        # reload otherwise lands on the m0 critical chain)
        powwarm = T([BJ, 1], "powwarm")
        nc.gpsimd.tensor_tensor(
            powwarm[:, :], halfs[:, 0:1], halfs[:, 0:1], ALU.pow
        )

        # ----- bf16 cast of x (+ones) on ACT/Pool; f32 m0 colsum on DVE -----
        for b in range(NB):
            if b % 2 == 0:
                nc.scalar.copy(x_bf[:, :, b, 0:D], x_main[:, b, :, :])
            else:
                nc.gpsimd.tensor_copy(x_bf[:, :, b, 0:D], x_main[:, b, :, :])
            nc.vector.reduce_sum(
                part0[:, b, :],
                x_main[:, b, :, :].transpose([0, 2, 1]),
                axis=mybir.AxisListType.X,
            )

        with tc.tile_pool(name="yp", bufs=2, space="PSUM") as yp:
            # Wr[(j,b), d, k] = W_aug[d, j*16+k] via selector matmul
            wr_ps = yp.tile([BJ, DA, KD], f32, tag="ypsum", name="wr_ps")
            nc.tensor.matmul(
                wr_ps[:, :, :], cJ10, W10[:, :, :], start=True, stop=True
            )
            nc.vector.tensor_copy(Wr[:, :, :], wr_ps[:, :, :])

            def s_and_squash(m, y_ps):
                """s, squash -> o; returns o_sb. f32 throughout.

                The masked mul + d-reduce run in two k-halves so the reduce
                of half 0 overlaps the mul of half 1 in the DVE pipe.
                """
                sBt = T([BJ, KD, KT], f"sBt_{m}")
                s_sb = T([BJ, KD], f"s_sb_{m}")
                nc.vector.tensor_tensor(
                    sBt[:, :, :],
                    y_ps[:, :].unsqueeze(1).broadcast_to((BJ, KD, KT)),
                    WrBIGt[:, :, :, :].rearrange("p k b d -> p k (b d)"),
                    ALU.mult,
                )
                nc.vector.reduce_sum(
                    s_sb[:, :], sBt[:, :, :], axis=mybir.AxisListType.X
                )
                s2 = T([BJ, KD], f"s2_{m}")
                nc.vector.tensor_tensor(s2[:, :], s_sb[:, :], s_sb[:, :], ALU.mult)
                nsq_ps = yp.tile([BJ, KD], f32, tag="ypsum", name=f"nsq_{m}")
                nc.tensor.matmul(
                    nsq_ps[:, :], cB80, s2[:, :], start=True, stop=True
                )
                nsq_sb = T([BJ, KD], f"nsq_sb_{m}")
                nc.vector.tensor_scalar_add(nsq_sb[:, :], nsq_ps[:, :], 1e-12)
                u = T([BJ, KD], f"u_{m}")
                nc.gpsimd.tensor_tensor(
                    u[:, :], nsq_sb[:, :], halfs[:, :], ALU.pow
                )
                d1 = T([BJ, KD], f"d1_{m}")
                nc.vector.tensor_scalar_add(d1[:, :], nsq_sb[:, :], 1.0)
                dd = T([BJ, KD], f"dd_{m}")
                nc.vector.tensor_tensor(dd[:, :], d1[:, :], u[:, :], ALU.mult)
                rr = T([BJ, KD], f"rr_{m}")
                nc.vector.reciprocal_approx_fast(rr[:, :], dd[:, :])
                sc = T([BJ, KD], f"sc_{m}")
                nc.vector.tensor_tensor(sc[:, :], nsq_sb[:, :], rr[:, :], ALU.mult)
                o_sb = T([BJ, KD], f"o_{m}")
                nc.vector.tensor_tensor(o_sb[:, :], s_sb[:, :], sc[:, :], ALU.mult)
                return o_sb

            def vhat_update(m, o_sb):
                """vacc (+)= Wr . o; build blkv (hi/lo bf16)."""
                vt = T([BJ, DA, KD], f"vt_{m}")
                nc.vector.tensor_tensor(
                    vt[:, :, :],
                    o_sb[:, :].unsqueeze(1).broadcast_to((BJ, DA, KD)),
                    Wr[:, :, :], ALU.mult,
                )
                if m == 0:
                    nc.vector.reduce_sum(
                        vacc[:, :], vt[:, :, :], axis=mybir.AxisListType.X
                    )
                else:
                    v_cur = T([BJ, DA], f"v_cur_{m}")
                    nc.vector.reduce_sum(
                        v_cur[:, :], vt[:, :, :], axis=mybir.AxisListType.X
                    )
                    nc.vector.tensor_tensor(
                        vacc[:, :], vacc[:, :], v_cur[:, :], ALU.add
                    )
                vT_ps = yp.tile([DA, BJ], f32, tag="ypsum", name=f"vT_{m}")
                nc.tensor.transpose(vT_ps[:, :], vacc[:, :], cI80)
                vT_sb = T([DA, BJ], f"vT_sb_{m}")
                nc.vector.tensor_copy(vT_sb[:, :], vT_ps[:, :])
                vdup_ps = yp.tile([KT, BJ], f32, tag="ypsum", name=f"vd_{m}")
                nc.tensor.matmul(
                    vdup_ps[:, :], cREP, vT_sb[:, :], start=True, stop=True
                )
                blkM = T([KT, BJ], f"blkM_{m}")
                nc.vector.tensor_tensor(
                    blkM[:, :], cBLK, vdup_ps[:, :], ALU.mult
                )
                blkv_n = blkv_t[m]
                nc.vector.tensor_copy(blkv_n[:, 0, :], blkM[:, :])
                nc.gpsimd.tensor_sub(
                    blkv_n[:, 1, :], blkM[:, :], blkv_n[:, 0, :]
                )
                if _DEBUG_TAPS:
                    nc.sync.dma_start(dbg_d[m], vacc[:, :])

            # ================= m = 0 (uniform c shortcut, f32) =================
            y0r_ps = yp.tile([1, NB * D], f32, tag="ypsum", name="y0r")
            nc.tensor.matmul(
                y0r_ps[:, :], onesF[:, :], part0[:, :, :], start=True, stop=True
            )
            nc.vector.tensor_copy(
                y0row[:, :].rearrange("p (b d) -> p b d", d=DA)[:, :, 0:D],
                y0r_ps[:, :].rearrange("p (b d) -> p b d", d=D),
            )
            nc.vector.memset(
                y0row[:, :].rearrange("p (b d) -> p b d", d=DA)[:, :, D],
                float(IN),
            )
            # WrBIGt[(j,b), k, (b',d)] = cBLKY * Wr (emitted after the y0row
            # ops so the m0 chain isn't queued behind it on DVE)
            nc.vector.tensor_tensor(
                WrBIGt[:, :, :, :],
                cBLKY.rearrange("p (b d) -> p b d", d=DA)
                .unsqueeze(1).broadcast_to((BJ, KD, NB, DA)),
                Wr[:, :, :].transpose([0, 2, 1])
                .unsqueeze(2).broadcast_to((BJ, KD, NB, DA)),
                ALU.mult,
            )
            y_ps0 = yp.tile([BJ, KT], f32, tag="ypsum", name="y_0")
            nc.tensor.matmul(
                y_ps0[:, :], tenth80[:, :], y0row[:, :], start=True, stop=True
            )
            if _DEBUG_TAPS:
                nc.sync.dma_start(dbg2_d[0, :, 0:DA * KD], Wr[:, :, :])
                nc.sync.dma_start(
                    dbg2_d[1],
                    WrBIGt[:, :, :, :].rearrange("p k b d -> p (k b d)"),
                )
                ydbg = T([BJ, KT], "ydbg")
                nc.vector.tensor_copy(ydbg[:, :], y_ps0[:, :])
                nc.sync.dma_start(dbg2_d[2, :, 0:KT], ydbg[:, :])
                nc.sync.dma_start(
                    dbg2_d[3, :, 0:CPACK_W], cpack[0:BJ, :]
                )
            o0 = s_and_squash(0, y_ps0)
            vhat_update(0, o0)

            # ----- xT build: 64 PE transposes, copies in 8-chunk groups -----
            with tc.tile_pool(name="tpp", bufs=4, space="PSUM") as tpp:
                for w in range(0, NH, 8):
                    tp = tpp.tile([KT, 8, P], bf16, tag="tp", name=f"tp_{w}")
                    for q in range(8):
                        nc.tensor.transpose(
                            tp[:, q, :], x_bf[:, w + q, :, :], cIbf[:, :]
                        )
                    if (w // 8) % 2 == 1:
                        nc.scalar.copy(xT[:, w:w + 8, :], tp[:, :, :])
                    else:
                        nc.vector.tensor_copy(xT[:, w:w + 8, :], tp[:, :, :])

            # ================= m = 1, 2 =================
            with tc.tile_pool(name="bwp", bufs=3, space="PSUM") as bwp:
                for m in (1, 2):
                    blkv = blkv_t[m - 1]
                    e = e_st[m]
                    cst = c_st[m]
                    u5, v2, w1 = u5_t[m], v2_t[m], w1_t[m]
                    Z, Zr, Zrb = Z_t[m], Zr_t[m], Zrb_t[m]

                    y_ps = yp.tile([BJ, KT], f32, tag="ypsum", name=f"y_{m}")

                    for h0, QH in ((0, 24), (24, 24), (48, 8), (56, 8)):
                        # --- b-logit waves: 8 (or 4) chunks each ---
                        for w0 in range(0, QH, 8):
                            wn = min(8, QH - w0)
                            wh = wn // 2
                            bw = bwp.tile(
                                [P, 2, 512], f32,
                                tag="bw", name=f"bw_{m}_{h0}_{w0}",
                            )
                            for c in range(wn):
                                h = h0 + w0 + c
                                off = (c % wh) * BJ
                                dst = bw[:, c // wh, off:off + BJ]
                                nc.tensor.matmul(
                                    dst, xT[:, h, :], blkv[:, 0, :],
                                    start=True, stop=False,
                                )
                                nc.tensor.matmul(
                                    dst, xT[:, h, :], blkv[:, 1, :],
                                    start=False, stop=True,
                                )
                            # exp -> e[p, h, j, b] (h-outer, contiguous)
                            hw0 = h0 + w0
                            nc.scalar.activation(
                                e[:, hw0:hw0 + wn, :, :]
                                .rearrange("p (a c) j b -> p a c (j b)", a=2),
                                bw[:, :, 0:wh * BJ]
                                .rearrange("p a (c x) -> p a c x", x=BJ),
                                AF.Exp,
                            )
                        hs = slice(h0, h0 + QH)
                        # --- Z = sum_j e via bf16 pair tree (DVE 2x) ---
                        nc.vector.tensor_tensor(
                            u5[:, hs, :, :], e[:, hs, 0:5, :], e[:, hs, 5:10, :],
                            ALU.add,
                        )
                        nc.vector.tensor_tensor(
                            v2[:, hs, :, :], u5[:, hs, 0:2, :], u5[:, hs, 2:4, :],
                            ALU.add,
                        )
                        nc.vector.tensor_tensor(
                            w1[:, hs, :], v2[:, hs, 0, :], v2[:, hs, 1, :],
                            ALU.add,
                        )
                        nc.vector.tensor_tensor(
                            Z[:, hs, :], w1[:, hs, :], u5[:, hs, 4, :], ALU.add
                        )
                        nc.vector.reciprocal_approx_fast(
                            Zr[:, hs, :].rearrange("p h b -> p (h b)"),
                            Z[:, hs, :].rearrange("p h b -> p (h b)"),
                        )
                        nc.vector.tensor_copy(Zrb[:, hs, :], Zr[:, hs, :])
                        # --- c = e * Zr (outer-dim broadcast keeps DVE 2x) ---
                        nc.vector.tensor_tensor(
                            cst[:, hs, :, :], e[:, hs, :, :],
                            Zrb[:, hs, :].unsqueeze(2)
                            .broadcast_to((P, QH, J, NB)),
                            ALU.mult,
                        )
                        # --- y accumulation for this quarter ---
                        for h in range(h0, h0 + QH):
                            nc.tensor.matmul(
                                y_ps[:, :],
                                cst[:, h, :, :],
                                x_bf[:, h, :, :],
                                start=(h == 0), stop=(h == NH - 1),
                            )

                    o_sb = s_and_squash(m, y_ps)

                    if m < 2:
                        vhat_update(m, o_sb)
                    else:
                        # ---- final lengths ||o[(j,b), :]|| over k ----
                        osq = T([BJ, KD], "osq")
                        lsum = T([BJ, 1], "lsum")
                        nc.vector.tensor_tensor(
                            osq[:, :], o_sb[:, :], o_sb[:, :], ALU.mult
                        )
                        nc.vector.reduce_sum(
                            lsum[:, :], osq[:, :], axis=mybir.AxisListType.X
                        )
                        lnorm = T([BJ, 1], "lnorm")
                        nc.gpsimd.tensor_tensor(
                            lnorm[:, :], lsum[:, :], halfs[:, 0:1], ALU.pow
                        )
                        nc.sync.dma_start(out_d[:, :], lnorm[:, :])

    nc.compile()
    return nc


_NC_CACHE = None


def _get_nc():
    global _NC_CACHE
    if _NC_CACHE is None:
        _NC_CACHE = _build_nc()
    return _NC_CACHE


def kernel(x, W, bias):
    x = np.ascontiguousarray(np.asarray(x, dtype=np.float32))
    W = np.ascontiguousarray(np.asarray(W, dtype=np.float32))
    bias = np.ascontiguousarray(np.asarray(bias, dtype=np.float32))
    B = x.shape[0]
    per = B // N_CORES

    nc = _get_nc()
    in_maps = [
        {"x": x[i * per:(i + 1) * per], "W": W, "bias": bias}
        for i in range(N_CORES)
    ]
    res = bass_utils.run_bass_kernel_spmd(
        nc, in_maps, core_ids=list(range(N_CORES))
    )
    # rows are (j, b): out[j*8+b] -> [b, j]
    outs = [r["out"].reshape(J, NB).T for r in res.results]
    return np.concatenate(outs, axis=0)


if __name__ == "__main__":
    rng = np.random.default_rng(0)
    x = rng.standard_normal((64, IN, D), dtype=np.float32)
    W = (rng.standard_normal((D, J * KD)) / np.sqrt(D)).astype(np.float32)
    bias = (rng.standard_normal(J * KD) * 0.01).astype(np.float32)
    out = kernel(x=x, W=W, bias=bias)
    print(out.shape, out[0])


# revision 71
# speedup vs baseline: 1.5594x; 1.0154x over previous
"""DenseCapsule routing kernel for Trainium2 (Bass/Tile), 8-core data-parallel.

Problem: x [64, 8192, 8], W [8, 160], bias [160] ->
  x_hat = (x @ W + bias).reshape(64, 8192, 10, 16)
  3 dynamic-routing iterations (softmax over out_num=10, weighted sum over
  in_num=8192, squash over the 10-axis, agreement update), return
  ||outputs||_2 over out_dim -> [64, 10].

Key algebra (x_hat is never materialized):
  s[b,j,:]  = y[b,j,:] @ Wr[:,j,:]      with y = c^T @ x_aug  (tiny [10,9])
  b_logits  = x_aug @ vhat_acc^T        vhat accumulates over iterations
  softmax over j=10: c = e * (1/Z), Z via a bf16 pair-tree
  sqrt via gpsimd pow so ACT stays on ONE function table (exp only).

Precision: x_aug, c in single bf16; vhat_acc split hi/lo bf16 and the two
halves accumulated in PSUM by two back-to-back matmuls (f32 accumulate);
m0 colsum and the s/squash/vhat path in f32. End-to-end ~7e-4 rel err.

Sharding: batch 64 -> 8 cores x 8 batches. Row space for [80, *] tensors is
(j, b) = j*8+b; xT rows are (b, d) = b*9+d.
"""

from contextlib import ExitStack

import numpy as np

import concourse.bacc as bacc
import concourse.bass as bass
import concourse.mybir as mybir
import concourse.tile as tile
import concourse.bass_utils as bass_utils

f32 = mybir.dt.float32
bf16 = mybir.dt.bfloat16
AF = mybir.ActivationFunctionType
ALU = mybir.AluOpType

_DEBUG_TAPS = False

P = 128          # SBUF partitions
NH = 64          # i-chunks per batch (8192 / 128)
NB = 8           # batches per core
D = 8            # input capsule dim
DA = 9           # augmented (+ ones column)
J = 10           # out_num
KD = 16          # out_dim
KT = NB * DA     # 72 rows (b, d)
BJ = NB * J      # 80 rows (j, b) = j*8+b
IN = 8192
N_CORES = 8


def _build_nc():
    nc = bacc.Bacc(
        "TRN2", target_bir_lowering=False, debug=False, num_devices=N_CORES
    )

    x_d = nc.dram_tensor("x", [NB, IN, D], f32, kind="ExternalInput").ap()
    w_d = nc.dram_tensor("W", [D, J * KD], f32, kind="ExternalInput").ap()
    bias_d = nc.dram_tensor("bias", [J * KD], f32, kind="ExternalInput").ap()
    out_d = nc.dram_tensor("out", [BJ, 1], f32, kind="ExternalOutput").ap()
    dbg_d = None
    dbg2_d = None
    if _DEBUG_TAPS:
        dbg_d = nc.dram_tensor(
            "dbg", [2, BJ, DA], f32, kind="ExternalOutput"
        ).ap()
        dbg2_d = nc.dram_tensor(
            "dbg2", [4, BJ, KD * KT], f32, kind="ExternalOutput"
        ).ap()

    # ---- structural constants, packed into one f32 block + bf16 identity ----
    import ml_dtypes
    identbf_np = np.eye(P, dtype=np.float32).astype(ml_dtypes.bfloat16)

    # column offsets in the packed f32 const block
    C_I80, C_B80, C_BLK, C_BLKY, C_J10, C_REP = 0, 80, 160, 240, 312, 392
    CPACK_W = C_REP + KT
    cpack_np = np.zeros((P, CPACK_W), dtype=np.float32)
    cpack_np[0:BJ, C_I80:C_I80 + BJ] = np.eye(BJ, dtype=np.float32)
    for j in range(J):
        for b in range(NB):
            for j2 in range(J):
                # cB80[(j,b), (j',b')] = 1 iff b == b'
                cpack_np[j * NB + b, C_B80 + j2 * NB + b] = 1.0
    for b in range(NB):
        for d in range(DA):
            for j in range(J):
                # cBLK[(b,d), (j,b')] = 1 iff b == b'
                cpack_np[b * DA + d, C_BLK + j * NB + b] = 1.0
    for j in range(J):
        for b in range(NB):
            # cBLKY[(j,b), (b',d)] = 1 iff b' == b
            cpack_np[j * NB + b, C_BLKY + b * DA:C_BLKY + (b + 1) * DA] = 1.0
    for j in range(J):
        for b in range(NB):
            # cJ10[j', (j,b)] = 1 iff j' == j
            cpack_np[j, C_J10 + j * NB + b] = 1.0
    for b in range(NB):
        for d in range(DA):
            # cREP[d, (b,d')] = 1 iff d' == d
            cpack_np[d, C_REP + b * DA + d] = 1.0

    identbf_d = nc.inline_tensor(identbf_np, "identbf").ap()
    cpack_d = nc.inline_tensor(cpack_np, "cpack").ap()

    with tile.TileContext(nc) as tc, ExitStack() as ctx:
        sbp = ctx.enter_context(tc.tile_pool(name="sbp", bufs=1))

        def T(shape, name, dt=f32):
            return sbp.tile(shape, dt, name=name, tag=name)

        # ----- persistent SBUF tensors -----
        x_main = T([P, NB, NH, D], "x_main")          # raw DMA staging
        x_bf = T([P, NH, NB, DA], "x_bf", bf16)       # bf16 x_aug (ones col)
        xT = T([KT, NH, P], "xT", bf16)               # x_aug^T per chunk
        cIbf = T([P, P], "cIbf", bf16)                # identity (transposes)
        cpack = T([P, CPACK_W], "cpack")              # packed f32 constants
        cI80 = cpack[0:BJ, C_I80:C_I80 + BJ]
        cB80 = cpack[0:BJ, C_B80:C_B80 + BJ]
        cBLK = cpack[0:KT, C_BLK:C_BLK + BJ]
        cBLKY = cpack[0:BJ, C_BLKY:C_BLKY + KT]
        cJ10 = cpack[0:J, C_J10:C_J10 + BJ]
        cREP = cpack[0:DA, C_REP:C_REP + KT]
        W10 = T([J, DA, KD], "W10")                   # W_aug per j
        Wr = T([BJ, DA, KD], "Wr")                    # Wr[(j,b), d, k]
        WrBIGt = T([BJ, KD, NB, DA], "WrBIGt")        # masked, k-outer (f32)
        vacc = T([BJ, DA], "vacc")
        halfs = T([BJ, KD], "halfs")                  # 0.5 (gpsimd pow -> sqrt)
        part0 = T([P, NB, D], "part0")                # m0 f32 colsum partials
        onesF = T([P, 1], "onesF")                    # f32 ones column
        tenth80 = T([1, BJ], "tenth80")               # 0.1 expander row
        y0row = T([1, KT], "y0row")                   # m0 colsum row

        e_st = [None, T([P, NH, J, NB], "e1", bf16), T([P, NH, J, NB], "e2", bf16)]
        c_st = [None, T([P, NH, J, NB], "c1", bf16), T([P, NH, J, NB], "c2", bf16)]
        u5_t = [None, T([P, NH, 5, NB], "u5_1", bf16), T([P, NH, 5, NB], "u5_2", bf16)]
        v2_t = [None, T([P, NH, 2, NB], "v2_1", bf16), T([P, NH, 2, NB], "v2_2", bf16)]
        w1_t = [None, T([P, NH, NB], "w1_1", bf16), T([P, NH, NB], "w1_2", bf16)]
        Z_t = [None, T([P, NH, NB], "Z_1"), T([P, NH, NB], "Z_2")]
        Zr_t = [None, T([P, NH, NB], "Zr_1"), T([P, NH, NB], "Zr_2")]
        Zrb_t = [None, T([P, NH, NB], "Zrb_1", bf16), T([P, NH, NB], "Zrb_2", bf16)]
        blkv_t = [T([KT, 2, BJ], "blkv0", bf16), T([KT, 2, BJ], "blkv1", bf16)]

        # ----- input x: 8 contiguous per-batch DMAs over both HW queues -----
        for b in range(NB):
            eng = nc.sync if b % 2 == 0 else nc.scalar
            eng.dma_start(
                x_main[:, b, :, :],
                x_d[b].rearrange("(p h) d -> p h d", p=P),
            )

        # ----- constants -----
        nc.sync.dma_start(cpack[:, :], cpack_d[:, :])
        nc.scalar.dma_start(cIbf[:, :], identbf_d[:, :])
        # W10[j, d, k] = W_aug[d, j*16+k]
        nc.scalar.dma_start(
            W10[:, 0:D, :],
            bass.AP(tensor=w_d.tensor, offset=0,
                    ap=[[KD, J], [J * KD, D], [1, KD]]),
        )
        nc.scalar.dma_start(
            W10[:, D, :],
            bass.AP(tensor=bias_d.tensor, offset=0, ap=[[KD, J], [1, KD]]),
        )

        nc.gpsimd.memset(halfs[:, :], 0.5)
        nc.gpsimd.memset(onesF[:, :], 1.0)
        nc.gpsimd.memset(tenth80[:, :], 1.0 / J)
        nc.gpsimd.memset(x_bf[:, :, :, D], 1.0)
        # warm the gpsimd pow library during the DMA wait (the ext-isa
        # reload otherwise lands on the m0 critical chain)
        powwarm = T([BJ, 1], "powwarm")
        nc.gpsimd.tensor_tensor(
            powwarm[:, :], halfs[:, 0:1], halfs[:, 0:1], ALU.pow
        )

        # ----- bf16 cast of x (+ones) on ACT/Pool; f32 m0 colsum on DVE -----
        for b in range(NB):
            if b % 2 == 0:
                nc.scalar.copy(x_bf[:, :, b, 0:D], x_main[:, b, :, :])
            else:
                nc.gpsimd.tensor_copy(x_bf[:, :, b, 0:D], x_main[:, b, :, :])
            nc.vector.reduce_sum(
                part0[:, b, :],
                x_main[:, b, :, :].transpose([0, 2, 1]),
                axis=mybir.AxisListType.X,
            )

        with tc.tile_pool(name="yp", bufs=2, space="PSUM") as yp:
            # Wr[(j,b), d, k] = W_aug[d, j*16+k] via selector matmul
            wr_ps = yp.tile([BJ, DA, KD], f32, tag="ypsum", name="wr_ps")
            nc.tensor.matmul(
                wr_ps[:, :, :], cJ10, W10[:, :, :], start=True, stop=True
            )
            nc.vector.tensor_copy(Wr[:, :, :], wr_ps[:, :, :])

            def s_and_squash(m, y_ps):
                """s, squash -> o; returns o_sb. f32 throughout.

                The masked mul + d-reduce run in two k-halves so the reduce
                of half 0 overlaps the mul of half 1 in the DVE pipe.
                """
                sBt = T([BJ, KD, KT], f"sBt_{m}")
                s_sb = T([BJ, KD], f"s_sb_{m}")
                nc.vector.tensor_tensor(
                    sBt[:, :, :],
                    y_ps[:, :].unsqueeze(1).broadcast_to((BJ, KD, KT)),
                    WrBIGt[:, :, :, :].rearrange("p k b d -> p k (b d)"),
                    ALU.mult,
                )
                nc.vector.reduce_sum(
                    s_sb[:, :], sBt[:, :, :], axis=mybir.AxisListType.X
                )
                s2 = T([BJ, KD], f"s2_{m}")
                nc.vector.tensor_tensor(s2[:, :], s_sb[:, :], s_sb[:, :], ALU.mult)
                nsq_ps = yp.tile([BJ, KD], f32, tag="ypsum", name=f"nsq_{m}")
                nc.tensor.matmul(
                    nsq_ps[:, :], cB80, s2[:, :], start=True, stop=True
                )
                nsq_sb = T([BJ, KD], f"nsq_sb_{m}")
                nc.vector.tensor_scalar_add(nsq_sb[:, :], nsq_ps[:, :], 1e-12)
                u = T([BJ, KD], f"u_{m}")
                nc.gpsimd.tensor_tensor(
                    u[:, :], nsq_sb[:, :], halfs[:, :], ALU.pow
                )
                d1 = T([BJ, KD], f"d1_{m}")
                nc.vector.tensor_scalar_add(d1[:, :], nsq_sb[:, :], 1.0)
                dd = T([BJ, KD], f"dd_{m}")
                nc.vector.tensor_tensor(dd[:, :], d1[:, :], u[:, :], ALU.mult)
                rr = T([BJ, KD], f"rr_{m}")
                nc.vector.reciprocal_approx_fast(rr[:, :], dd[:, :])
                sc = T([BJ, KD], f"sc_{m}")
                nc.vector.tensor_tensor(sc[:, :], nsq_sb[:, :], rr[:, :], ALU.mult)
                o_sb = T([BJ, KD], f"o_{m}")
                nc.vector.tensor_tensor(o_sb[:, :], s_sb[:, :], sc[:, :], ALU.mult)
                return o_sb

            def vhat_update(m, o_sb):
                """vacc (+)= Wr . o; build blkv (hi/lo bf16)."""
                vt = T([BJ, DA, KD], f"vt_{m}")
                nc.vector.tensor_tensor(
                    vt[:, :, :],
                    o_sb[:, :].unsqueeze(1).broadcast_to((BJ, DA, KD)),
                    Wr[:, :, :], ALU.mult,
                )
                if m == 0:
                    nc.vector.reduce_sum(
                        vacc[:, :], vt[:, :, :], axis=mybir.AxisListType.X
                    )
                else:
                    v_cur = T([BJ, DA], f"v_cur_{m}")
                    nc.vector.reduce_sum(
                        v_cur[:, :], vt[:, :, :], axis=mybir.AxisListType.X
                    )
                    nc.vector.tensor_tensor(
                        vacc[:, :], vacc[:, :], v_cur[:, :], ALU.add
                    )
                vT_ps = yp.tile([DA, BJ], f32, tag="ypsum", name=f"vT_{m}")
                nc.tensor.transpose(vT_ps[:, :], vacc[:, :], cI80)
                vT_sb = T([DA, BJ], f"vT_sb_{m}")
                nc.vector.tensor_copy(vT_sb[:, :], vT_ps[:, :])
                vdup_ps = yp.tile([KT, BJ], f32, tag="ypsum", name=f"vd_{m}")
                nc.tensor.matmul(
                    vdup_ps[:, :], cREP, vT_sb[:, :], start=True, stop=True
                )
                blkM = T([KT, BJ], f"blkM_{m}")
                nc.vector.tensor_tensor(
                    blkM[:, :], cBLK, vdup_ps[:, :], ALU.mult
                )
                blkv_n = blkv_t[m]
                nc.vector.tensor_copy(blkv_n[:, 0, :], blkM[:, :])
                nc.gpsimd.tensor_sub(
                    blkv_n[:, 1, :], blkM[:, :], blkv_n[:, 0, :]
                )
                if _DEBUG_TAPS:
                    nc.sync.dma_start(dbg_d[m], vacc[:, :])

            # ================= m = 0 (uniform c shortcut, f32) =================
            y0r_ps = yp.tile([1, NB * D], f32, tag="ypsum", name="y0r")
            nc.tensor.matmul(
                y0r_ps[:, :], onesF[:, :], part0[:, :, :], start=True, stop=True
            )
            nc.vector.tensor_copy(
                y0row[:, :].rearrange("p (b d) -> p b d", d=DA)[:, :, 0:D],
                y0r_ps[:, :].rearrange("p (b d) -> p b d", d=D),
            )
            nc.vector.memset(
                y0row[:, :].rearrange("p (b d) -> p b d", d=DA)[:, :, D],
                float(IN),
            )
            # WrBIGt[(j,b), k, (b',d)] = cBLKY * Wr (emitted after the y0row
            # ops so the m0 chain isn't queued behind it on DVE)
            nc.vector.tensor_tensor(
                WrBIGt[:, :, :, :],
                cBLKY.rearrange("p (b d) -> p b d", d=DA)
                .unsqueeze(1).broadcast_to((BJ, KD, NB, DA)),
                Wr[:, :, :].transpose([0, 2, 1])
                .unsqueeze(2).broadcast_to((BJ, KD, NB, DA)),
                ALU.mult,
            )
            y_ps0 = yp.tile([BJ, KT], f32, tag="ypsum", name="y_0")
            nc.tensor.matmul(
                y_ps0[:, :], tenth80[:, :], y0row[:, :], start=True, stop=True
            )
            if _DEBUG_TAPS:
                nc.sync.dma_start(dbg2_d[0, :, 0:DA * KD], Wr[:, :, :])
                nc.sync.dma_start(
                    dbg2_d[1],
                    WrBIGt[:, :, :, :].rearrange("p k b d -> p (k b d)"),
                )
                ydbg = T([BJ, KT], "ydbg")
                nc.vector.tensor_copy(ydbg[:, :], y_ps0[:, :])
                nc.sync.dma_start(dbg2_d[2, :, 0:KT], ydbg[:, :])
                nc.sync.dma_start(
                    dbg2_d[3, :, 0:CPACK_W], cpack[0:BJ, :]
                )
            o0 = s_and_squash(0, y_ps0)
            vhat_update(0, o0)

            # ----- xT build: 64 PE transposes, copies in 8-chunk groups -----
            with tc.tile_pool(name="tpp", bufs=4, space="PSUM") as tpp:
                for w in range(0, NH, 8):
                    tp = tpp.tile([KT, 8, P], bf16, tag="tp", name=f"tp_{w}")
                    for q in range(8):
                        nc.tensor.transpose(
                            tp[:, q, :], x_bf[:, w + q, :, :], cIbf[:, :]
                        )
                    if (w // 8) % 2 == 1:
                        nc.scalar.copy(xT[:, w:w + 8, :], tp[:, :, :])
                    else:
                        nc.vector.tensor_copy(xT[:, w:w + 8, :], tp[:, :, :])

            # ================= m = 1, 2 =================
            with tc.tile_pool(name="bwp", bufs=3, space="PSUM") as bwp:
                for m in (1, 2):
                    blkv = blkv_t[m - 1]
                    e = e_st[m]
                    cst = c_st[m]
                    u5, v2, w1 = u5_t[m], v2_t[m], w1_t[m]
                    Z, Zr, Zrb = Z_t[m], Zr_t[m], Zrb_t[m]

                    y_ps = yp.tile([BJ, KT], f32, tag="ypsum", name=f"y_{m}")

                    for h0, QH in ((0, 24), (24, 24), (48, 8), (56, 8)):
                        # --- b-logit waves: 8 (or 4) chunks each ---
                        for w0 in range(0, QH, 8):
                            wn = min(8, QH - w0)
                            wh = wn // 2
                            bw = bwp.tile(
                                [P, 2, 512], f32,
                                tag="bw", name=f"bw_{m}_{h0}_{w0}",
                            )
                            for c in range(wn):
                                h = h0 + w0 + c
                                off = (c % wh) * BJ
                                dst = bw[:, c // wh, off:off + BJ]
                                nc.tensor.matmul(
                                    dst, xT[:, h, :], blkv[:, 0, :],
                                    start=True, stop=False,
                                )
                                nc.tensor.matmul(
                                    dst, xT[:, h, :], blkv[:, 1, :],
                                    start=False, stop=True,
                                )
                            # exp -> e[p, h, j, b] (h-outer, contiguous)
                            hw0 = h0 + w0
                            nc.scalar.activation(
                                e[:, hw0:hw0 + wn, :, :]
                                .rearrange("p (a c) j b -> p a c (j b)", a=2),
                                bw[:, :, 0:wh * BJ]
                                .rearrange("p a (c x) -> p a c x", x=BJ),
                                AF.Exp,
                            )
                        hs = slice(h0, h0 + QH)
                        # --- Z = sum_j e via bf16 pair tree (DVE 2x) ---
                        nc.vector.tensor_tensor(
                            u5[:, hs, :, :], e[:, hs, 0:5, :], e[:, hs, 5:10, :],
                            ALU.add,
                        )
                        nc.vector.tensor_tensor(
                            v2[:, hs, :, :], u5[:, hs, 0:2, :], u5[:, hs, 2:4, :],
                            ALU.add,
                        )
                        nc.vector.tensor_tensor(
                            w1[:, hs, :], v2[:, hs, 0, :], v2[:, hs, 1, :],
                            ALU.add,
                        )
                        nc.vector.tensor_tensor(
                            Z[:, hs, :], w1[:, hs, :], u5[:, hs, 4, :], ALU.add
                        )
                        nc.vector.reciprocal_approx_fast(
                            Zr[:, hs, :].rearrange("p h b -> p (h b)"),
                            Z[:, hs, :].rearrange("p h b -> p (h b)"),
                        )
                        nc.vector.tensor_copy(Zrb[:, hs, :], Zr[:, hs, :])
                        # --- c = e * Zr (outer-dim broadcast keeps DVE 2x);
                        # j 0:6 on DVE, 6:10 on Pool ---
                        nc.vector.tensor_tensor(
                            cst[:, hs, 0:6, :], e[:, hs, 0:6, :],
                            Zrb[:, hs, :].unsqueeze(2)
                            .broadcast_to((P, QH, 6, NB)),
                            ALU.mult,
                        )
                        nc.gpsimd.tensor_mul(
                            cst[:, hs, 6:10, :], e[:, hs, 6:10, :],
                            Zrb[:, hs, :].unsqueeze(2)
                            .broadcast_to((P, QH, 4, NB)),
                        )
                        # --- y accumulation for this quarter ---
                        for h in range(h0, h0 + QH):
                            nc.tensor.matmul(
                                y_ps[:, :],
                                cst[:, h, :, :],
                                x_bf[:, h, :, :],
                                start=(h == 0), stop=(h == NH - 1),
                            )

                    o_sb = s_and_squash(m, y_ps)

                    if m < 2:
                        vhat_update(m, o_sb)
                    else:
                        # ---- final lengths ||o[(j,b), :]|| over k ----
                        osq = T([BJ, KD], "osq")
                        lsum = T([BJ, 1], "lsum")
                        nc.vector.tensor_tensor(
                            osq[:, :], o_sb[:, :], o_sb[:, :], ALU.mult
                        )
                        nc.vector.reduce_sum(
                            lsum[:, :], osq[:, :], axis=mybir.AxisListType.X
                        )
                        lnorm = T([BJ, 1], "lnorm")
                        nc.gpsimd.tensor_tensor(
                            lnorm[:, :], lsum[:, :], halfs[:, 0:1], ALU.pow
                        )
                        nc.sync.dma_start(out_d[:, :], lnorm[:, :])

    nc.compile()
    return nc


_NC_CACHE = None


def _get_nc():
    global _NC_CACHE
    if _NC_CACHE is None:
        _NC_CACHE = _build_nc()
    return _NC_CACHE


def kernel(x, W, bias):
    x = np.ascontiguousarray(np.asarray(x, dtype=np.float32))
    W = np.ascontiguousarray(np.asarray(W, dtype=np.float32))
    bias = np.ascontiguousarray(np.asarray(bias, dtype=np.float32))
    B = x.shape[0]
    per = B // N_CORES

    nc = _get_nc()
    in_maps = [
        {"x": x[i * per:(i + 1) * per], "W": W, "bias": bias}
        for i in range(N_CORES)
    ]
    res = bass_utils.run_bass_kernel_spmd(
        nc, in_maps, core_ids=list(range(N_CORES))
    )
    # rows are (j, b): out[j*8+b] -> [b, j]
    outs = [r["out"].reshape(J, NB).T for r in res.results]
    return np.concatenate(outs, axis=0)


if __name__ == "__main__":
    rng = np.random.default_rng(0)
    x = rng.standard_normal((64, IN, D), dtype=np.float32)
    W = (rng.standard_normal((D, J * KD)) / np.sqrt(D)).astype(np.float32)
    bias = (rng.standard_normal(J * KD) * 0.01).astype(np.float32)
    out = kernel(x=x, W=W, bias=bias)
    print(out.shape, out[0])


# revision 72
# speedup vs baseline: 1.5704x; 1.0071x over previous
"""DenseCapsule routing kernel for Trainium2 (Bass/Tile), 8-core data-parallel.

Problem: x [64, 8192, 8], W [8, 160], bias [160] ->
  x_hat = (x @ W + bias).reshape(64, 8192, 10, 16)
  3 dynamic-routing iterations (softmax over out_num=10, weighted sum over
  in_num=8192, squash over the 10-axis, agreement update), return
  ||outputs||_2 over out_dim -> [64, 10].

Key algebra (x_hat is never materialized):
  s[b,j,:]  = y[b,j,:] @ Wr[:,j,:]      with y = c^T @ x_aug  (tiny [10,9])
  b_logits  = x_aug @ vhat_acc^T        vhat accumulates over iterations
  softmax over j=10: c = e * (1/Z), Z via a bf16 pair-tree
  sqrt via gpsimd pow so ACT stays on ONE function table (exp only).

Precision: x_aug, c in single bf16; vhat_acc split hi/lo bf16 and the two
halves accumulated in PSUM by two back-to-back matmuls (f32 accumulate);
m0 colsum and the s/squash/vhat path in f32. End-to-end ~7e-4 rel err.

Sharding: batch 64 -> 8 cores x 8 batches. Row space for [80, *] tensors is
(j, b) = j*8+b; xT rows are (b, d) = b*9+d.
"""

from contextlib import ExitStack

import numpy as np

import concourse.bacc as bacc
import concourse.bass as bass
import concourse.mybir as mybir
import concourse.tile as tile
import concourse.bass_utils as bass_utils

f32 = mybir.dt.float32
bf16 = mybir.dt.bfloat16
AF = mybir.ActivationFunctionType
ALU = mybir.AluOpType

_DEBUG_TAPS = False

P = 128          # SBUF partitions
NH = 64          # i-chunks per batch (8192 / 128)
NB = 8           # batches per core
D = 8            # input capsule dim
DA = 9           # augmented (+ ones column)
J = 10           # out_num
KD = 16          # out_dim
KT = NB * DA     # 72 rows (b, d)
BJ = NB * J      # 80 rows (j, b) = j*8+b
IN = 8192
N_CORES = 8


def _build_nc():
    nc = bacc.Bacc(
        "TRN2", target_bir_lowering=False, debug=False, num_devices=N_CORES
    )

    x_d = nc.dram_tensor("x", [NB, IN, D], f32, kind="ExternalInput").ap()
    w_d = nc.dram_tensor("W", [D, J * KD], f32, kind="ExternalInput").ap()
    bias_d = nc.dram_tensor("bias", [J * KD], f32, kind="ExternalInput").ap()
    out_d = nc.dram_tensor("out", [BJ, 1], f32, kind="ExternalOutput").ap()
    dbg_d = None
    dbg2_d = None
    if _DEBUG_TAPS:
        dbg_d = nc.dram_tensor(
            "dbg", [2, BJ, DA], f32, kind="ExternalOutput"
        ).ap()
        dbg2_d = nc.dram_tensor(
            "dbg2", [4, BJ, KD * KT], f32, kind="ExternalOutput"
        ).ap()

    # ---- structural constants, packed into one f32 block + bf16 identity ----
    import ml_dtypes
    identbf_np = np.eye(P, dtype=np.float32).astype(ml_dtypes.bfloat16)

    # column offsets in the packed f32 const block
    C_I80, C_B80, C_BLK, C_BLKY, C_J10, C_REP = 0, 80, 160, 240, 312, 392
    CPACK_W = C_REP + KT
    cpack_np = np.zeros((P, CPACK_W), dtype=np.float32)
    cpack_np[0:BJ, C_I80:C_I80 + BJ] = np.eye(BJ, dtype=np.float32)
    for j in range(J):
        for b in range(NB):
            for j2 in range(J):
                # cB80[(j,b), (j',b')] = 1 iff b == b'
                cpack_np[j * NB + b, C_B80 + j2 * NB + b] = 1.0
    for b in range(NB):
        for d in range(DA):
            for j in range(J):
                # cBLK[(b,d), (j,b')] = 1 iff b == b'
                cpack_np[b * DA + d, C_BLK + j * NB + b] = 1.0
    for j in range(J):
        for b in range(NB):
            # cBLKY[(j,b), (b',d)] = 1 iff b' == b
            cpack_np[j * NB + b, C_BLKY + b * DA:C_BLKY + (b + 1) * DA] = 1.0
    for j in range(J):
        for b in range(NB):
            # cJ10[j', (j,b)] = 1 iff j' == j
            cpack_np[j, C_J10 + j * NB + b] = 1.0
    for b in range(NB):
        for d in range(DA):
            # cREP[d, (b,d')] = 1 iff d' == d
            cpack_np[d, C_REP + b * DA + d] = 1.0

    identbf_d = nc.inline_tensor(identbf_np, "identbf").ap()
    cpack_d = nc.inline_tensor(cpack_np, "cpack").ap()

    with tile.TileContext(nc) as tc, ExitStack() as ctx:
        sbp = ctx.enter_context(tc.tile_pool(name="sbp", bufs=1))

        def T(shape, name, dt=f32):
            return sbp.tile(shape, dt, name=name, tag=name)

        # ----- persistent SBUF tensors -----
        x_main = T([P, NB, NH, D], "x_main")          # raw DMA staging
        x_bf = T([P, NH, NB, DA], "x_bf", bf16)       # bf16 x_aug (ones col)
        xT = T([KT, NH, P], "xT", bf16)               # x_aug^T per chunk
        cIbf = T([P, P], "cIbf", bf16)                # identity (transposes)
        cpack = T([P, CPACK_W], "cpack")              # packed f32 constants
        cI80 = cpack[0:BJ, C_I80:C_I80 + BJ]
        cB80 = cpack[0:BJ, C_B80:C_B80 + BJ]
        cBLK = cpack[0:KT, C_BLK:C_BLK + BJ]
        cBLKY = cpack[0:BJ, C_BLKY:C_BLKY + KT]
        cJ10 = cpack[0:J, C_J10:C_J10 + BJ]
        cREP = cpack[0:DA, C_REP:C_REP + KT]
        W10 = T([J, DA, KD], "W10")                   # W_aug per j
        Wr = T([BJ, DA, KD], "Wr")                    # Wr[(j,b), d, k]
        WrBIGt = T([BJ, KD, NB, DA], "WrBIGt")        # masked, k-outer (f32)
        vacc = T([BJ, DA], "vacc")
        halfs = T([BJ, KD], "halfs")                  # 0.5 (gpsimd pow -> sqrt)
        part0 = T([P, NB, D], "part0")                # m0 f32 colsum partials
        onesF = T([P, 1], "onesF")                    # f32 ones column
        tenth80 = T([1, BJ], "tenth80")               # 0.1 expander row
        y0row = T([1, KT], "y0row")                   # m0 colsum row

        e_st = [None, T([P, NH, J, NB], "e1", bf16), T([P, NH, J, NB], "e2", bf16)]
        c_st = [None, T([P, NH, J, NB], "c1", bf16), T([P, NH, J, NB], "c2", bf16)]
        u5_t = [None, T([P, NH, 5, NB], "u5_1", bf16), T([P, NH, 5, NB], "u5_2", bf16)]
        v2_t = [None, T([P, NH, 2, NB], "v2_1", bf16), T([P, NH, 2, NB], "v2_2", bf16)]
        w1_t = [None, T([P, NH, NB], "w1_1", bf16), T([P, NH, NB], "w1_2", bf16)]
        Z_t = [None, T([P, NH, NB], "Z_1"), T([P, NH, NB], "Z_2")]
        Zr_t = [None, T([P, NH, NB], "Zr_1"), T([P, NH, NB], "Zr_2")]
        Zrb_t = [None, T([P, NH, NB], "Zrb_1", bf16), T([P, NH, NB], "Zrb_2", bf16)]
        blkv_t = [T([KT, 2, BJ], "blkv0", bf16), T([KT, 2, BJ], "blkv1", bf16)]

        # ----- input x: 8 contiguous per-batch DMAs over both HW queues -----
        for b in range(NB):
            eng = nc.sync if b % 2 == 0 else nc.scalar
            eng.dma_start(
                x_main[:, b, :, :],
                x_d[b].rearrange("(p h) d -> p h d", p=P),
            )

        # ----- constants -----
        nc.sync.dma_start(cpack[:, :], cpack_d[:, :])
        nc.scalar.dma_start(cIbf[:, :], identbf_d[:, :])
        # W10[j, d, k] = W_aug[d, j*16+k]
        nc.scalar.dma_start(
            W10[:, 0:D, :],
            bass.AP(tensor=w_d.tensor, offset=0,
                    ap=[[KD, J], [J * KD, D], [1, KD]]),
        )
        nc.scalar.dma_start(
            W10[:, D, :],
            bass.AP(tensor=bias_d.tensor, offset=0, ap=[[KD, J], [1, KD]]),
        )

        nc.gpsimd.memset(halfs[:, :], 0.5)
        nc.gpsimd.memset(onesF[:, :], 1.0)
        nc.gpsimd.memset(tenth80[:, :], 1.0 / J)
        nc.gpsimd.memset(x_bf[:, :, :, D], 1.0)
        # warm the gpsimd pow library during the DMA wait (the ext-isa
        # reload otherwise lands on the m0 critical chain)
        powwarm = T([BJ, 1], "powwarm")
        nc.gpsimd.tensor_tensor(
            powwarm[:, :], halfs[:, 0:1], halfs[:, 0:1], ALU.pow
        )

        # ----- bf16 cast of x (+ones) on ACT/Pool; f32 m0 colsum on DVE -----
        for b in range(NB):
            if b % 2 == 0:
                nc.scalar.copy(x_bf[:, :, b, 0:D], x_main[:, b, :, :])
            else:
                nc.gpsimd.tensor_copy(x_bf[:, :, b, 0:D], x_main[:, b, :, :])
            nc.vector.reduce_sum(
                part0[:, b, :],
                x_main[:, b, :, :].transpose([0, 2, 1]),
                axis=mybir.AxisListType.X,
            )

        with tc.tile_pool(name="yp", bufs=2, space="PSUM") as yp:
            # Wr[(j,b), d, k] = W_aug[d, j*16+k] via selector matmul
            wr_ps = yp.tile([BJ, DA, KD], f32, tag="ypsum", name="wr_ps")
            nc.tensor.matmul(
                wr_ps[:, :, :], cJ10, W10[:, :, :], start=True, stop=True
            )
            nc.vector.tensor_copy(Wr[:, :, :], wr_ps[:, :, :])

            def s_and_squash(m, y_ps):
                """s, squash -> o; returns o_sb. f32 throughout.

                The masked mul + d-reduce run in two k-halves so the reduce
                of half 0 overlaps the mul of half 1 in the DVE pipe.
                """
                sBt = T([BJ, KD, KT], f"sBt_{m}")
                s_sb = T([BJ, KD], f"s_sb_{m}")
                nc.vector.tensor_tensor(
                    sBt[:, :, :],
                    y_ps[:, :].unsqueeze(1).broadcast_to((BJ, KD, KT)),
                    WrBIGt[:, :, :, :].rearrange("p k b d -> p k (b d)"),
                    ALU.mult,
                )
                nc.vector.reduce_sum(
                    s_sb[:, :], sBt[:, :, :], axis=mybir.AxisListType.X
                )
                s2 = T([BJ, KD], f"s2_{m}")
                nc.vector.tensor_tensor(s2[:, :], s_sb[:, :], s_sb[:, :], ALU.mult)
                nsq_ps = yp.tile([BJ, KD], f32, tag="ypsum", name=f"nsq_{m}")
                nc.tensor.matmul(
                    nsq_ps[:, :], cB80, s2[:, :], start=True, stop=True
                )
                nsq_sb = T([BJ, KD], f"nsq_sb_{m}")
                nc.vector.tensor_scalar_add(nsq_sb[:, :], nsq_ps[:, :], 1e-12)
                u = T([BJ, KD], f"u_{m}")
                nc.gpsimd.tensor_tensor(
                    u[:, :], nsq_sb[:, :], halfs[:, :], ALU.pow
                )
                dd = T([BJ, KD], f"dd_{m}")
                nc.vector.scalar_tensor_tensor(
                    dd[:, :], nsq_sb[:, :], 1.0, u[:, :], ALU.add, ALU.mult
                )
                rr = T([BJ, KD], f"rr_{m}")
                nc.vector.reciprocal_approx_fast(rr[:, :], dd[:, :])
                sc = T([BJ, KD], f"sc_{m}")
                nc.vector.tensor_tensor(sc[:, :], nsq_sb[:, :], rr[:, :], ALU.mult)
                o_sb = T([BJ, KD], f"o_{m}")
                nc.vector.tensor_tensor(o_sb[:, :], s_sb[:, :], sc[:, :], ALU.mult)
                return o_sb

            def vhat_update(m, o_sb):
                """vacc (+)= Wr . o; build blkv (hi/lo bf16)."""
                vt = T([BJ, DA, KD], f"vt_{m}")
                nc.vector.tensor_tensor(
                    vt[:, :, :],
                    o_sb[:, :].unsqueeze(1).broadcast_to((BJ, DA, KD)),
                    Wr[:, :, :], ALU.mult,
                )
                if m == 0:
                    nc.vector.reduce_sum(
                        vacc[:, :], vt[:, :, :], axis=mybir.AxisListType.X
                    )
                else:
                    v_cur = T([BJ, DA], f"v_cur_{m}")
                    nc.vector.reduce_sum(
                        v_cur[:, :], vt[:, :, :], axis=mybir.AxisListType.X
                    )
                    nc.vector.tensor_tensor(
                        vacc[:, :], vacc[:, :], v_cur[:, :], ALU.add
                    )
                vT_ps = yp.tile([DA, BJ], f32, tag="ypsum", name=f"vT_{m}")
                nc.tensor.transpose(vT_ps[:, :], vacc[:, :], cI80)
                vT_sb = T([DA, BJ], f"vT_sb_{m}")
                nc.vector.tensor_copy(vT_sb[:, :], vT_ps[:, :])
                vdup_ps = yp.tile([KT, BJ], f32, tag="ypsum", name=f"vd_{m}")
                nc.tensor.matmul(
                    vdup_ps[:, :], cREP, vT_sb[:, :], start=True, stop=True
                )
                blkM = T([KT, BJ], f"blkM_{m}")
                nc.vector.tensor_tensor(
                    blkM[:, :], cBLK, vdup_ps[:, :], ALU.mult
                )
                blkv_n = blkv_t[m]
                nc.vector.tensor_copy(blkv_n[:, 0, :], blkM[:, :])
                nc.gpsimd.tensor_sub(
                    blkv_n[:, 1, :], blkM[:, :], blkv_n[:, 0, :]
                )
                if _DEBUG_TAPS:
                    nc.sync.dma_start(dbg_d[m], vacc[:, :])

            # ================= m = 0 (uniform c shortcut, f32) =================
            y0r_ps = yp.tile([1, NB * D], f32, tag="ypsum", name="y0r")
            nc.tensor.matmul(
                y0r_ps[:, :], onesF[:, :], part0[:, :, :], start=True, stop=True
            )
            nc.vector.tensor_copy(
                y0row[:, :].rearrange("p (b d) -> p b d", d=DA)[:, :, 0:D],
                y0r_ps[:, :].rearrange("p (b d) -> p b d", d=D),
            )
            nc.vector.memset(
                y0row[:, :].rearrange("p (b d) -> p b d", d=DA)[:, :, D],
                float(IN),
            )
            # WrBIGt[(j,b), k, (b',d)] = cBLKY * Wr (emitted after the y0row
            # ops so the m0 chain isn't queued behind it on DVE)
            nc.vector.tensor_tensor(
                WrBIGt[:, :, :, :],
                cBLKY.rearrange("p (b d) -> p b d", d=DA)
                .unsqueeze(1).broadcast_to((BJ, KD, NB, DA)),
                Wr[:, :, :].transpose([0, 2, 1])
                .unsqueeze(2).broadcast_to((BJ, KD, NB, DA)),
                ALU.mult,
            )
            y_ps0 = yp.tile([BJ, KT], f32, tag="ypsum", name="y_0")
            nc.tensor.matmul(
                y_ps0[:, :], tenth80[:, :], y0row[:, :], start=True, stop=True
            )
            if _DEBUG_TAPS:
                nc.sync.dma_start(dbg2_d[0, :, 0:DA * KD], Wr[:, :, :])
                nc.sync.dma_start(
                    dbg2_d[1],
                    WrBIGt[:, :, :, :].rearrange("p k b d -> p (k b d)"),
                )
                ydbg = T([BJ, KT], "ydbg")
                nc.vector.tensor_copy(ydbg[:, :], y_ps0[:, :])
                nc.sync.dma_start(dbg2_d[2, :, 0:KT], ydbg[:, :])
                nc.sync.dma_start(
                    dbg2_d[3, :, 0:CPACK_W], cpack[0:BJ, :]
                )
            o0 = s_and_squash(0, y_ps0)
            vhat_update(0, o0)

            # ----- xT build: 64 PE transposes, copies in 8-chunk groups -----
            with tc.tile_pool(name="tpp", bufs=4, space="PSUM") as tpp:
                for w in range(0, NH, 8):
                    tp = tpp.tile([KT, 8, P], bf16, tag="tp", name=f"tp_{w}")
                    for q in range(8):
                        nc.tensor.transpose(
                            tp[:, q, :], x_bf[:, w + q, :, :], cIbf[:, :]
                        )
                    if (w // 8) % 2 == 1:
                        nc.scalar.copy(xT[:, w:w + 8, :], tp[:, :, :])
                    else:
                        nc.vector.tensor_copy(xT[:, w:w + 8, :], tp[:, :, :])

            # ================= m = 1, 2 =================
            with tc.tile_pool(name="bwp", bufs=3, space="PSUM") as bwp:
                for m in (1, 2):
                    blkv = blkv_t[m - 1]
                    e = e_st[m]
                    cst = c_st[m]
                    u5, v2, w1 = u5_t[m], v2_t[m], w1_t[m]
                    Z, Zr, Zrb = Z_t[m], Zr_t[m], Zrb_t[m]

                    y_ps = yp.tile([BJ, KT], f32, tag="ypsum", name=f"y_{m}")

                    for h0, QH in ((0, 24), (24, 24), (48, 8), (56, 8)):
                        # --- b-logit waves: 8 (or 4) chunks each ---
                        for w0 in range(0, QH, 8):
                            wn = min(8, QH - w0)
                            wh = wn // 2
                            bw = bwp.tile(
                                [P, 2, 512], f32,
                                tag="bw", name=f"bw_{m}_{h0}_{w0}",
                            )
                            for c in range(wn):
                                h = h0 + w0 + c
                                off = (c % wh) * BJ
                                dst = bw[:, c // wh, off:off + BJ]
                                nc.tensor.matmul(
                                    dst, xT[:, h, :], blkv[:, 0, :],
                                    start=True, stop=False,
                                )
                                nc.tensor.matmul(
                                    dst, xT[:, h, :], blkv[:, 1, :],
                                    start=False, stop=True,
                                )
                            # exp -> e[p, h, j, b] (h-outer, contiguous)
                            hw0 = h0 + w0
                            nc.scalar.activation(
                                e[:, hw0:hw0 + wn, :, :]
                                .rearrange("p (a c) j b -> p a c (j b)", a=2),
                                bw[:, :, 0:wh * BJ]
                                .rearrange("p a (c x) -> p a c x", x=BJ),
                                AF.Exp,
                            )
                        hs = slice(h0, h0 + QH)
                        # --- Z = sum_j e via bf16 pair tree (DVE 2x) ---
                        nc.vector.tensor_tensor(
                            u5[:, hs, :, :], e[:, hs, 0:5, :], e[:, hs, 5:10, :],
                            ALU.add,
                        )
                        nc.vector.tensor_tensor(
                            v2[:, hs, :, :], u5[:, hs, 0:2, :], u5[:, hs, 2:4, :],
                            ALU.add,
                        )
                        nc.vector.tensor_tensor(
                            w1[:, hs, :], v2[:, hs, 0, :], v2[:, hs, 1, :],
                            ALU.add,
                        )
                        nc.vector.tensor_tensor(
                            Z[:, hs, :], w1[:, hs, :], u5[:, hs, 4, :], ALU.add
                        )
                        nc.vector.reciprocal_approx_fast(
                            Zr[:, hs, :].rearrange("p h b -> p (h b)"),
                            Z[:, hs, :].rearrange("p h b -> p (h b)"),
                        )
                        nc.vector.tensor_copy(Zrb[:, hs, :], Zr[:, hs, :])
                        # --- c = e * Zr (outer-dim broadcast keeps DVE 2x);
                        # j 0:6 on DVE, 6:10 on Pool ---
                        nc.vector.tensor_tensor(
                            cst[:, hs, 0:6, :], e[:, hs, 0:6, :],
                            Zrb[:, hs, :].unsqueeze(2)
                            .broadcast_to((P, QH, 6, NB)),
                            ALU.mult,
                        )
                        nc.gpsimd.tensor_mul(
                            cst[:, hs, 6:10, :], e[:, hs, 6:10, :],
                            Zrb[:, hs, :].unsqueeze(2)
                            .broadcast_to((P, QH, 4, NB)),
                        )
                        # --- y accumulation for this quarter ---
                        for h in range(h0, h0 + QH):
                            nc.tensor.matmul(
                                y_ps[:, :],
                                cst[:, h, :, :],
                                x_bf[:, h, :, :],
                                start=(h == 0), stop=(h == NH - 1),
                            )

                    o_sb = s_and_squash(m, y_ps)

                    if m < 2:
                        vhat_update(m, o_sb)
                    else:
                        # ---- final lengths ||o[(j,b), :]|| over k ----
                        osq = T([BJ, KD], "osq")
                        lsum = T([BJ, 1], "lsum")
                        nc.vector.tensor_tensor(
                            osq[:, :], o_sb[:, :], o_sb[:, :], ALU.mult
                        )
                        nc.vector.reduce_sum(
                            lsum[:, :], osq[:, :], axis=mybir.AxisListType.X
                        )
                        lnorm = T([BJ, 1], "lnorm")
                        nc.gpsimd.tensor_tensor(
                            lnorm[:, :], lsum[:, :], halfs[:, 0:1], ALU.pow
                        )
                        nc.sync.dma_start(out_d[:, :], lnorm[:, :])

    nc.compile()
    return nc


_NC_CACHE = None


def _get_nc():
    global _NC_CACHE
    if _NC_CACHE is None:
        _NC_CACHE = _build_nc()
    return _NC_CACHE


def kernel(x, W, bias):
    x = np.ascontiguousarray(np.asarray(x, dtype=np.float32))
    W = np.ascontiguousarray(np.asarray(W, dtype=np.float32))
    bias = np.ascontiguousarray(np.asarray(bias, dtype=np.float32))
    B = x.shape[0]
    per = B // N_CORES

    nc = _get_nc()
    in_maps = [
        {"x": x[i * per:(i + 1) * per], "W": W, "bias": bias}
        for i in range(N_CORES)
    ]
    res = bass_utils.run_bass_kernel_spmd(
        nc, in_maps, core_ids=list(range(N_CORES))
    )
    # rows are (j, b): out[j*8+b] -> [b, j]
    outs = [r["out"].reshape(J, NB).T for r in res.results]
    return np.concatenate(outs, axis=0)


if __name__ == "__main__":
    rng = np.random.default_rng(0)
    x = rng.standard_normal((64, IN, D), dtype=np.float32)
    W = (rng.standard_normal((D, J * KD)) / np.sqrt(D)).astype(np.float32)
    bias = (rng.standard_normal(J * KD) * 0.01).astype(np.float32)
    out = kernel(x=x, W=W, bias=bias)
    print(out.shape, out[0])
